# revision 7
# baseline (speedup 1.0000x reference)
"""ColumnParallelLinearWithMoE Trainium2 kernel.

Expert-parallel: expert e -> NeuronCore e. Each core computes
    y_e [8192, 512] = x_e [8192, 1024] @ W_e.T [1024, 512] (+ b_e)
where x_e = input_[idx_list[e]] flattened over (per, seq).

Routing gather/scatter and all layout shuffles happen on the host; the
device does the dense matmul in bf16 (fp32 PSUM accumulation) and stores
bf16. The bias is zero in this module (zero-initialized); if a nonzero
bias ever shows up it is applied on the host during unshard, so the
device pipeline is matmul -> DVE copy (fp32->bf16) -> store.

Timing model (measured on HW):
- exec ~= (last_matmul_ts - 6.0us) + 11.1us: the clock starts at the
  first user instruction (~6us into the trace) and a fixed ~11us tail
  follows the last matmul (final evict+store drain ~3us + a fixed ~8us
  TileContext teardown that clears all 254 semaphores regardless of how
  many the kernel used).
- Body floor is 512 matmuls x 216ns = 110.6us (N=512 moving operand,
  warm PE at 2.4GHz). So the game is: start the warm matmul stream as
  early as possible.
- Head floor: engine preambles end ~7.2us (sync/scalar) and ~7.8us
  (tensor); w is 1MB = 2.8us of HBM transfer, so the k-accumulation of
  the first token tile finishes ~11.5us at best. A 5-matmul PE warmup
  from 7.8us makes HAM un-throttle (1.2->2.4GHz) by ~11.2us.
- DMA issue cost on the HWDGE rings scales with descriptor count
  (~3.5us for a 1024-descriptor 2MB load). All DRAM layouts here are
  host-shuffled so every transfer is contiguous per partition (128
  descriptors, ~0.6us issue).
- PSUM dependency tracking is tile-granular: per-j [128,512] PSUM tiles
  so each eviction waits only on its own 8-matmul accumulation group.
- The last super evicts/stores per token-tile so the post-matmul drain
  stays short.

Device-side layouts (p = SBUF partition = low 7 bits of the d/contraction
index, t = token-in-tile, j = token tile, k = contraction tile, s = super):
  xs [128, NSUP, TPS*KT*128]: xs[p, s, j*1024 + k*128 + t] = x[token, d]
  ws [128, KT*OPP]:           ws[p, k*512 + c] = W.T[d, c]
  ys [128, NSUP, TPS*OPP]:    ys[p, s, j*512 + c] = y[token, c]
with token = s*1024 + j*128 + t(=p for ys), d = k*128 + p.
"""

import sys

if "/opt/trn_rl_repo" not in sys.path:
    sys.path.insert(0, "/opt/trn_rl_repo")

import numpy as np

# Problem constants (hardcoded per harness contract).
E = 8
BS = 64
S = 1024
D = 1024
OPP = 512
P = 128
TOK = (BS // E) * S  # 8192 tokens per expert
KT = D // P          # 8 contraction tiles
TW = 1024            # token-superblock width staged in SBUF
NSUP = TOK // TW     # 8 superblocks
TPS = TW // P        # 8 token tiles (of 128) per superblock

N_WARM = 12          # warmup matmuls bridging the initial DMA window

_programs: dict[str, tuple] = {}


def _build():
    import concourse.bacc as bacc
    import concourse.tile as tile
    from concourse import mybir
    import ml_dtypes

    mm_dt = mybir.dt.bfloat16
    np_in = ml_dtypes.bfloat16

    nc = bacc.Bacc(None, target_bir_lowering=False, debug=False)

    xs = nc.dram_tensor("xs", [P, NSUP, TPS * KT * P], mm_dt, kind="ExternalInput")
    ws = nc.dram_tensor("ws", [P, KT * OPP], mm_dt, kind="ExternalInput")
    ys = nc.dram_tensor("ys", [P, NSUP, TPS * OPP], mm_dt, kind="ExternalOutput")

    with tile.TileContext(nc) as tc:
        with (
            tc.tile_pool(name="wpool", bufs=1) as wpool,
            tc.tile_pool(name="xpool", bufs=4) as xpool,
            tc.tile_pool(name="opool", bufs=3) as opool,
            tc.tile_pool(name="pspool", bufs=8, space="PSUM") as pspool,
        ):
            # Ramp: x token-chunks on the sync ring, w k-chunks on the
            # scalar ring, issuing in parallel from the moment each engine's
            # preamble ends. Every transfer is per-partition contiguous.
            x0_sb = xpool.tile([P, TPS * KT * P], mm_dt, tag="x")
            w_sb = wpool.tile([P, KT * OPP], mm_dt)
            CH = KT * P  # 1024 elements per 128-token chunk per partition
            # w goes FIRST on the sync ring: HWDGE ring order is effective
            # priority on the shared SDMA engines, and w's 1MB transfer
            # (~2.8us of HBM) is the critical path to the first full
            # k-accumulation. x0 token-chunks trickle in on the scalar
            # ring; x0 chunk j is only needed ~1.7us*j after the stream
            # starts. Superblock loads queue on sync AFTER w.
            nc.sync.dma_start(out=w_sb[:, 0 : 4 * OPP], in_=ws[:, 0 : 4 * OPP])
            nc.sync.dma_start(out=w_sb[:, 4 * OPP :], in_=ws[:, 4 * OPP :])
            for j in range(TPS):
                nc.scalar.dma_start(
                    out=x0_sb[:, j * CH : (j + 1) * CH],
                    in_=xs[:, 0, j * CH : (j + 1) * CH],
                )

            # PE warmup on a zeroed tile: keeps the PE busy from ~7.8us
            # (end of the tensor-engine preamble) so HAM un-throttles by
            # ~11.2us, right when the real stream reaches full rate.
            warm_src = wpool.tile([P, OPP], mm_dt, tag="warm")
            nc.gpsimd.memset(warm_src[:], 0.0)
            warm_ps = pspool.tile([P, OPP], mybir.dt.float32, tag="ps")
            for _ in range(N_WARM):
                nc.tensor.matmul(
                    warm_ps[:], warm_src[:, :P], warm_src[:], start=True, stop=True
                )

            for s in range(NSUP):
                if s == 0:
                    x_sb = x0_sb
                else:
                    x_sb = xpool.tile([P, TPS * KT * P], mm_dt, tag="x")
                    nc.sync.dma_start(out=x_sb[:], in_=xs[:, s, :])
                o_sb = opool.tile([P, TPS * OPP], mm_dt, tag="o")
                last_s = s == NSUP - 1
                for j in range(TPS):
                    ps = pspool.tile([P, OPP], mybir.dt.float32, tag="ps")
                    for k in range(KT):
                        nc.tensor.matmul(
                            ps[:],
                            x_sb[:, j * CH + k * P : j * CH + (k + 1) * P],
                            w_sb[:, k * OPP : (k + 1) * OPP],
                            start=(k == 0),
                            stop=(k == KT - 1),
                        )
                    nc.vector.tensor_copy(o_sb[:, j * OPP : (j + 1) * OPP], ps[:])
                    if last_s:
                        # Fine-grained tail: store each token tile as soon
                        # as it is evicted.
                        nc.scalar.dma_start(
                            out=ys[:, s, j * OPP : (j + 1) * OPP],
                            in_=o_sb[:, j * OPP : (j + 1) * OPP],
                        )
                if not last_s:
                    nc.scalar.dma_start(out=ys[:, s, :], in_=o_sb[:])

    nc.compile()
    return nc, np_in


def _get_program():
    if "v4" not in _programs:
        _programs["v4"] = _build()
    return _programs["v4"]


def kernel(input_, idx_list, W, b, **_ignored):
    from concourse.bass_utils import run_bass_kernel_spmd

    input_ = np.asarray(input_)
    idx = np.asarray(idx_list).astype(np.int64)
    W = np.asarray(W, dtype=np.float32)
    b = np.asarray(b, dtype=np.float32)

    nc, np_in = _get_program()

    in_maps = []
    for e in range(E):
        xg = input_[idx[e]].reshape(TOK, D)
        # xs[p, s, j*1024 + k*128 + t] = x[s*1024 + j*128 + t, k*128 + p]
        xhost = np.ascontiguousarray(
            xg.reshape(NSUP, TPS, P, KT, P).transpose(4, 0, 1, 3, 2)
        ).reshape(P, NSUP, TPS * KT * P)
        # ws[p, k*512 + c] = W[c, k*128 + p]
        whost = np.ascontiguousarray(
            W[e].reshape(OPP, KT, P).transpose(2, 1, 0)
        ).reshape(P, KT * OPP)
        in_maps.append(
            {"xs": xhost.astype(np_in), "ws": whost.astype(np_in)}
        )

    res = run_bass_kernel_spmd(nc, in_maps, core_ids=list(range(E)))

    out = np.zeros((BS, S, E * OPP), dtype=input_.dtype)
    for e in range(E):
        yd = np.asarray(res.results[e]["ys"]).astype(input_.dtype)
        # ys[p, s, j*512 + c] -> y[s*1024 + j*128 + p, c]
        ye = yd.reshape(P, NSUP, TPS, OPP).transpose(1, 2, 0, 3).reshape(
            BS // E, S, OPP
        )
        if b[e].any():
            ye = ye + b[e][None, None, :]
        out[idx[e], :, e * OPP : (e + 1) * OPP] = ye
    return out


# revision 8
# speedup vs baseline: 1.0795x; 1.0795x over previous
"""ColumnParallelLinearWithMoE Trainium2 kernel.

Expert-parallel: expert e -> NeuronCore e. Each core computes
    y_e [8192, 512] = x_e [8192, 1024] @ W_e.T [1024, 512] (+ b_e)
where x_e = input_[idx_list[e]] flattened over (per, seq).

Routing gather/scatter and all layout shuffles happen on the host; the
device does the dense matmul in bf16 (fp32 PSUM accumulation) and stores
bf16. The bias is zero in this module (zero-initialized); if a nonzero
bias ever shows up it is applied on the host during unshard, so the
device pipeline is matmul -> DVE copy (fp32->bf16) -> store.

Timing model (measured on HW):
- exec ~= (last_matmul_ts - 6.0us) + 11.1us: the clock starts at the
  first user instruction (~6us into the trace) and a fixed ~11us tail
  follows the last matmul (final evict+store drain ~3us + a fixed ~8us
  TileContext teardown that clears all 254 semaphores regardless of how
  many the kernel used).
- Body floor is 512 matmuls x 216ns = 110.6us (N=512 moving operand,
  warm PE at 2.4GHz). So the game is: start the warm matmul stream as
  early as possible.
- Head floor: engine preambles end ~7.2us (sync/scalar) and ~7.8us
  (tensor); w is 1MB = 2.8us of HBM transfer, so the k-accumulation of
  the first token tile finishes ~11.5us at best. A 5-matmul PE warmup
  from 7.8us makes HAM un-throttle (1.2->2.4GHz) by ~11.2us.
- DMA issue cost on the HWDGE rings scales with descriptor count
  (~3.5us for a 1024-descriptor 2MB load). All DRAM layouts here are
  host-shuffled so every transfer is contiguous per partition (128
  descriptors, ~0.6us issue).
- PSUM dependency tracking is tile-granular: per-j [128,512] PSUM tiles
  so each eviction waits only on its own 8-matmul accumulation group.
- The last super evicts/stores per token-tile so the post-matmul drain
  stays short.

Device-side layouts (p = SBUF partition = low 7 bits of the d/contraction
index, t = token-in-tile, j = token tile, k = contraction tile, s = super):
  xs [128, NSUP, TPS*KT*128]: xs[p, s, j*1024 + k*128 + t] = x[token, d]
  ws [128, KT*OPP]:           ws[p, k*512 + c] = W.T[d, c]
  ys [128, NSUP, TPS*OPP]:    ys[p, s, j*512 + c] = y[token, c]
with token = s*1024 + j*128 + t(=p for ys), d = k*128 + p.
"""

import sys

if "/opt/trn_rl_repo" not in sys.path:
    sys.path.insert(0, "/opt/trn_rl_repo")

import numpy as np

# Problem constants (hardcoded per harness contract).
E = 8
BS = 64
S = 1024
D = 1024
OPP = 512
P = 128
TOK = (BS // E) * S  # 8192 tokens per expert
KT = D // P          # 8 contraction tiles
TW = 1024            # token-superblock width staged in SBUF
NSUP = TOK // TW     # 8 superblocks
TPS = TW // P        # 8 token tiles (of 128) per superblock

N_WARM = 10          # warmup matmuls bridging the initial DMA window

_programs: dict[str, tuple] = {}


def _build():
    import concourse.bacc as bacc
    import concourse.tile as tile
    from concourse import mybir
    import ml_dtypes

    mm_dt = mybir.dt.bfloat16
    np_in = ml_dtypes.bfloat16

    nc = bacc.Bacc(None, target_bir_lowering=False, debug=False)

    xs = nc.dram_tensor("xs", [P, NSUP, TPS * KT * P], mm_dt, kind="ExternalInput")
    ws = nc.dram_tensor("ws", [P, KT * OPP], mm_dt, kind="ExternalInput")
    ys = nc.dram_tensor("ys", [P, NSUP, TPS * OPP], mm_dt, kind="ExternalOutput")

    with tile.TileContext(nc) as tc:
        with (
            tc.tile_pool(name="wpool", bufs=1) as wpool,
            tc.tile_pool(name="xpool", bufs=4) as xpool,
            tc.tile_pool(name="opool", bufs=3) as opool,
            tc.tile_pool(name="pspool", bufs=8, space="PSUM") as pspool,
        ):
            # Ramp: x token-chunks on the sync ring, w k-chunks on the
            # scalar ring, issuing in parallel from the moment each engine's
            # preamble ends. Every transfer is per-partition contiguous.
            x0_sb = xpool.tile([P, TPS * KT * P], mm_dt, tag="x")
            w_sb = wpool.tile([P, KT * OPP], mm_dt)
            CH = KT * P  # 1024 elements per 128-token chunk per partition
            # All loads go on ONE ring (sync) in consumption order: the two
            # HWDGE rings round-robin on the shared SDMA engines, so a
            # second ring gives fair interleaving, not priority -- but
            # within a ring, FIFO order IS priority. Consume order:
            # w[0:4] (first half of the k-sweep), x0 chunk 0, w[4:8],
            # then the remaining token chunks; superblock loads queue
            # behind in the s-loop. Stores live on the scalar ring.
            nc.sync.dma_start(out=w_sb[:, 0 : 4 * OPP], in_=ws[:, 0 : 4 * OPP])
            nc.sync.dma_start(out=x0_sb[:, 0:CH], in_=xs[:, 0, 0:CH])
            nc.sync.dma_start(out=w_sb[:, 4 * OPP :], in_=ws[:, 4 * OPP :])
            for j in range(1, TPS):
                nc.sync.dma_start(
                    out=x0_sb[:, j * CH : (j + 1) * CH],
                    in_=xs[:, 0, j * CH : (j + 1) * CH],
                )

            # PE warmup on a zeroed tile: keeps the PE busy from ~7.8us
            # (end of the tensor-engine preamble) so HAM un-throttles by
            # ~11.2us, right when the real stream reaches full rate.
            warm_src = wpool.tile([P, OPP], mm_dt, tag="warm")
            nc.gpsimd.memset(warm_src[:], 0.0)
            warm_ps = pspool.tile([P, OPP], mybir.dt.float32, tag="ps")
            for _ in range(N_WARM):
                nc.tensor.matmul(
                    warm_ps[:], warm_src[:, :P], warm_src[:], start=True, stop=True
                )

            for s in range(NSUP):
                if s == 0:
                    x_sb = x0_sb
                else:
                    x_sb = xpool.tile([P, TPS * KT * P], mm_dt, tag="x")
                    nc.sync.dma_start(out=x_sb[:], in_=xs[:, s, :])
                o_sb = opool.tile([P, TPS * OPP], mm_dt, tag="o")
                last_s = s == NSUP - 1
                for j in range(TPS):
                    ps = pspool.tile([P, OPP], mybir.dt.float32, tag="ps")
                    for k in range(KT):
                        nc.tensor.matmul(
                            ps[:],
                            x_sb[:, j * CH + k * P : j * CH + (k + 1) * P],
                            w_sb[:, k * OPP : (k + 1) * OPP],
                            start=(k == 0),
                            stop=(k == KT - 1),
                        )
                    nc.vector.tensor_copy(o_sb[:, j * OPP : (j + 1) * OPP], ps[:])
                    if last_s:
                        # Fine-grained tail: store each token tile as soon
                        # as it is evicted.
                        nc.scalar.dma_start(
                            out=ys[:, s, j * OPP : (j + 1) * OPP],
                            in_=o_sb[:, j * OPP : (j + 1) * OPP],
                        )
                if not last_s:
                    nc.scalar.dma_start(out=ys[:, s, :], in_=o_sb[:])

    nc.compile()
    return nc, np_in


def _get_program():
    if "v4" not in _programs:
        _programs["v4"] = _build()
    return _programs["v4"]


def kernel(input_, idx_list, W, b, **_ignored):
    from concourse.bass_utils import run_bass_kernel_spmd

    input_ = np.asarray(input_)
    idx = np.asarray(idx_list).astype(np.int64)
    W = np.asarray(W, dtype=np.float32)
    b = np.asarray(b, dtype=np.float32)

    nc, np_in = _get_program()

    in_maps = []
    for e in range(E):
        xg = input_[idx[e]].reshape(TOK, D)
        # xs[p, s, j*1024 + k*128 + t] = x[s*1024 + j*128 + t, k*128 + p]
        xhost = np.ascontiguousarray(
            xg.reshape(NSUP, TPS, P, KT, P).transpose(4, 0, 1, 3, 2)
        ).reshape(P, NSUP, TPS * KT * P)
        # ws[p, k*512 + c] = W[c, k*128 + p]
        whost = np.ascontiguousarray(
            W[e].reshape(OPP, KT, P).transpose(2, 1, 0)
        ).reshape(P, KT * OPP)
        in_maps.append(
            {"xs": xhost.astype(np_in), "ws": whost.astype(np_in)}
        )

    res = run_bass_kernel_spmd(nc, in_maps, core_ids=list(range(E)))

    out = np.zeros((BS, S, E * OPP), dtype=input_.dtype)
    for e in range(E):
        yd = np.asarray(res.results[e]["ys"]).astype(input_.dtype)
        # ys[p, s, j*512 + c] -> y[s*1024 + j*128 + p, c]
        ye = yd.reshape(P, NSUP, TPS, OPP).transpose(1, 2, 0, 3).reshape(
            BS // E, S, OPP
        )
        if b[e].any():
            ye = ye + b[e][None, None, :]
        out[idx[e], :, e * OPP : (e + 1) * OPP] = ye
    return out
